# revision 3
# baseline (speedup 1.0000x reference)
"""BandSplit (gather -> per-band MLP -> scatter-add OLA -> /ola) on 8 TRN2 cores.

Strategy
--------
The whole reference computation is linear in x: fold everything into one
block-banded matrix A of shape (C*F, C*F) so that per (b, t) token
out = A^T vec(x) + const (const == 0 here; added on host regardless).
Data-parallel over the 4096 (b, t) tokens across 8 cores, 512 tokens each,
no cross-core communication.

v2 layout (vs the 128-row/channel-split v1):
 - Contraction chunks are 64 f-rows x 2 channels interleaved on the 128
   partitions (p = 2*fl + ci).  Halving the row span shrinks each chunk's
   output window (the band support), cutting total matmul columns from
   ~27.8k to ~19.1k cycles, and both input channels ride one matmul.
 - The packed band matrix `ab` ships as fp8 e3m4 scaled by 2^6, with the
   2^-6 compensation folded into the host-side bf16 cast of x.  PSUM then
   holds true-scale outputs; drains are plain f32->f16 copies.
 - Loads alternate between the two HWDGE rings (ACT: ab + even xs chunks,
   SP: odd xs chunks) so the load phase approaches the HBM per-core limit.
 - A short junk-matmul burst bridges PE-boot -> first-data and latches the
   free-running HAM clock-gate at 2.4 GHz before the real stream begins.
 - Stream is chunk-major matching DMA arrival order; PSUM bank lifetimes
   are 2-colored per token chunk (8 banks total).  Each (bank, tch) piece
   drains (DVE/ACT alternating) and stores as soon as its last chunk ends.
"""

import numpy as np

_P = 128
_C = 2
_F = 1025
_R = 64                     # f-rows per contraction chunk (x2 ci = 128)
_NJ = 17                    # chunks: j<16 cover f in [64j,64j+64); j=16 -> f=1024
_TCH = 4                    # token chunks (of 128) per core
_TCORE = _TCH * _P          # 512 tokens per core
_PS_W = _C * _F             # 2050 output columns (col = 2*fo + co)
_BANKS = [(b * 512, min(_PS_W, (b + 1) * 512)) for b in range((_PS_W + 511) // 512)]
_SCALE_BITS = 6             # ab * 2^6 in fp8; x * 2^-6 in bf16


def _fold_matrix(pre_w, pre_b, post_w, post_b, idx, melw, mask, ola_window):
    """Fold the full reference computation into (A, const).

    A: (C, F, C, F) with out[co, fo] = sum_{ci, fi} x[ci, fi] * A[ci, fi, co, fo]
    const: (C, F) additive constant from the biases.
    """
    K, W = idx.shape
    C = _C
    F = ola_window.shape[0]

    pre_w = np.asarray(pre_w, np.float64)
    post_w = np.asarray(post_w, np.float64)
    pre_b = np.asarray(pre_b, np.float64)
    post_b = np.asarray(post_b, np.float64)
    wts = (np.asarray(melw, np.float64) * np.asarray(mask, np.float64))
    msk = np.asarray(mask, np.float64)
    idx = np.asarray(idx)

    M = np.einsum('kio,koj->kij', pre_w, post_w).reshape(K, W, C, W, C)
    vals = M * wts[:, :, None, None, None] * msk[:, None, None, :, None]

    fin = idx[:, :, None, None, None].astype(np.int64)
    fout = idx[:, None, None, :, None].astype(np.int64)
    cin = np.arange(C)[None, None, :, None, None]
    cout = np.arange(C)[None, None, None, None, :]
    flat = ((cin * F + fin) * C + cout) * F + fout
    A = np.bincount(
        np.broadcast_to(flat, vals.shape).ravel(), weights=vals.ravel(),
        minlength=C * F * C * F,
    ).reshape(C, F, C, F)
    A /= ola_window[None, None, None, :]

    bv = (np.einsum('ko,koj->kj', pre_b, post_w) + post_b).reshape(K, W, C)
    bv = bv * msk[:, :, None]
    cflat = (np.arange(C)[None, None, :] * F + idx[:, :, None]).astype(np.int64)
    const = np.bincount(
        np.broadcast_to(cflat, bv.shape).ravel(), weights=bv.ravel(),
        minlength=C * F,
    ).reshape(C, F)
    const /= ola_window[None, :]
    return A, const


def _plan(A):
    """Windows, packed offsets, bank touch lists and PSUM slot colors."""
    # window per chunk over the co-interleaved output columns (2*fo+co)
    wins = []                   # j -> (lo, hi) in fo units
    for j in range(_NJ):
        f0, f1 = j * _R, min((j + 1) * _R, _F)
        blk = A[:, f0:f1, :, :]                      # (ci, rows, co, fo)
        cols = (blk != 0).any(axis=(0, 1, 2))
        nzc = np.nonzero(cols)[0]
        assert len(nzc) > 0
        wins.append((int(nzc[0]), int(nzc[-1]) + 1))
    covered = np.zeros(_F, bool)
    for lo, hi in wins:
        covered[lo:hi] = True
    assert covered.all(), "window coverage hole"

    offs = {}
    tw = 0
    for j in range(_NJ):
        offs[j] = tw
        tw += (2 * (wins[j][1] - wins[j][0]) + 15) // 16 * 16

    # per-bank (j, s, e) touches in stream order; s, e in column units
    touches = {}
    for j in range(_NJ):
        lo2, hi2 = 2 * wins[j][0], 2 * wins[j][1]
        for b, (bs, be) in enumerate(_BANKS):
            s, e = max(lo2, bs), min(hi2, be)
            if s < e:
                touches.setdefault(b, []).append((j, s, e))
    first_j = {b: t[0][0] for b, t in touches.items()}
    last_j = {b: t[-1][0] for b, t in touches.items()}

    # 2-color bank lifetimes: at any chunk j at most 2 banks live per tch
    colors = {}
    for b in sorted(touches):
        used = {colors[o] for o in colors
                if not (last_j[o] < first_j[b] or last_j[b] < first_j[o])}
        free = [c for c in "AB" if c not in used]
        assert free, f"PSUM slot coloring needs >2 colors at bank {b}"
        colors[b] = free[0]
    return wins, offs, tw, touches, first_j, last_j, colors


_PROGRAM_CACHE = {}


def _build_program(wins, offs, TW, touches, first_j, last_j, colors, n_cores):
    import concourse.tile as tile
    import concourse.mybir as mybir
    from concourse import bacc

    f32 = mybir.dt.float32
    bf16 = mybir.dt.bfloat16
    f16 = mybir.dt.float16
    f8e3 = mybir.dt.float8e3
    P = _P
    XCOLS = 16 * _TCORE          # 8192 cols: j*512 + tch*128 + tok  (j<16)
    W16 = 2 * (wins[16][1] - wins[16][0])

    nc = bacc.Bacc("TRN2", target_bir_lowering=False, debug=False,
                   num_devices=n_cores)
    # xs: bf16 * 2^-6, partition p = 2*fl + ci, col = j*512 + tch*128 + tok
    xs = nc.dram_tensor("xs", [P, XCOLS], bf16, kind="ExternalInput")
    # x1: the f=1024 row (chunk 16): row ci, col = tch*128 + tok
    x1 = nc.dram_tensor("x1", [2, _TCORE], bf16, kind="ExternalInput")
    # ab: packed fp8 band windows; chunk j at cols offs[j], col = 2*(fo-lo)+co
    ab = nc.dram_tensor("ab", [P, TW], f8e3, kind="ExternalInput")
    # y: f16, col = 2*fo + co, per token chunk
    y = nc.dram_tensor("y", [P, _TCH, _PS_W], f16, kind="ExternalOutput")

    with tile.TileContext(nc) as tc:
        with (
            tc.tile_pool(name="apool", bufs=1) as apool,
            tc.tile_pool(name="xpool", bufs=1) as xpool,
            tc.tile_pool(name="opool", bufs=1) as opool,
            tc.tile_pool(name="jpool", bufs=1) as jpool,
            tc.tile_pool(name="pspool", bufs=1, space="PSUM") as pspool,
        ):
            abig = apool.tile([P, TW], f8e3, name="abig")
            xbig = xpool.tile([P, XCOLS], bf16, name="xbig")
            t1 = jpool.tile([2, _TCORE], bf16, name="t1")
            junk = jpool.tile([P, P], bf16, name="junk")
            ot = opool.tile([P, _TCH, _PS_W], f16, name="ot")

            nc.vector.memset(junk[:], 0.0)

            # loads: ab_j + even xs_j on the ACT ring, odd xs_j + x1 on SP.
            # j-major order matches the stream's consumption order.
            nc.sync.dma_start(t1[:], x1[:])
            for j in range(16):
                o0, o1 = offs[j], offs[j] + 2 * (wins[j][1] - wins[j][0])
                nc.scalar.dma_start(abig[:, o0:o1], ab[:, o0:o1])
                eng = nc.scalar if j % 2 == 0 else nc.sync
                eng.dma_start(xbig[:, j * _TCORE:(j + 1) * _TCORE],
                              xs[:, j * _TCORE:(j + 1) * _TCORE])
            o0 = offs[16]
            nc.scalar.dma_start(abig[0:2, o0:o0 + W16], ab[0:2, o0:o0 + W16])

            # warmup burst: bridge PE-boot -> first data and latch the HAM
            # clock-gate window at 2.4 GHz before the stream begins.
            warm = pspool.tile([P, 512], f32, tag=f"{colors[0]}0", name="warm")
            for _ in range(26):
                nc.tensor.matmul(warm[:, :P], junk[:], junk[:],
                                 start=True, stop=True)

            cur = {}                    # (tch, b) -> PSUM tile
            drain_rr = [0]

            def drain(tch, b):
                bs, be = _BANKS[b]
                t = cur.pop((tch, b))
                if drain_rr[0] % 2 == 0:
                    nc.vector.tensor_copy(ot[:, tch, bs:be], t[:])
                else:
                    nc.scalar.copy(ot[:, tch, bs:be], t[:])
                drain_rr[0] += 1

            def emit_mms(j, tch):
                lo2 = 2 * wins[j][0]
                if j < 16:
                    lhsT = xbig[:, j * _TCORE + tch * P:j * _TCORE + (tch + 1) * P]
                else:
                    lhsT = t1[:, tch * P:(tch + 1) * P]
                o = offs[j]
                for b, (bs, be) in enumerate(_BANKS):
                    s, e = max(lo2, bs), min(2 * wins[j][1], be)
                    if s >= e:
                        continue
                    if (tch, b) not in cur:
                        cur[(tch, b)] = pspool.tile(
                            [P, be - bs], f32, tag=f"{colors[b]}{tch}",
                            name=f"bk{b}_{tch}")
                    order = touches[b]
                    if j == 16:
                        at = abig[0:2, o + s - lo2:o + e - lo2]
                    else:
                        at = abig[:, o + s - lo2:o + e - lo2]
                    nc.tensor.matmul(
                        cur[(tch, b)][:, s - bs:e - bs],
                        lhsT, at,
                        start=(order[0][0] == j), stop=(order[-1][0] == j),
                    )

            # store piece per (bank, tch) as soon as its drain is done; the
            # 2-col bank 4 piece merges into bank 3's store.
            def store(tch, b, eng):
                bs = _BANKS[b][0]
                be = _BANKS[b + 1][1] if b + 1 < len(_BANKS) and \
                    last_j.get(b + 1) == last_j[b] else _BANKS[b][1]
                eng.dma_start(y[:, tch, bs:be], ot[:, tch, bs:be])

            merged = {b for b in touches
                      if b > 0 and last_j.get(b - 1) == last_j[b]}

            for j in range(15):
                for tch in range(_TCH):
                    emit_mms(j, tch)
                for b in sorted(touches):
                    if last_j[b] == j:
                        for tch in range(_TCH):
                            drain(tch, b)
                        if b not in merged:
                            for tch in range(_TCH):
                                store(tch, b, nc.sync)

            # tail: per token chunk, the last two chunks + final drains and
            # stores overlap the later chunks' matmuls
            tail_banks = sorted(b for b in touches if last_j[b] >= 15)
            for tch in range(_TCH):
                emit_mms(15, tch)
                emit_mms(16, tch)
                for b in tail_banks:
                    drain(tch, b)
                for b in tail_banks:
                    if b not in merged:
                        store(tch, b, nc.scalar if tch % 2 else nc.sync)

    nc.compile()
    return nc


def kernel(**inputs):
    import ml_dtypes

    x = np.ascontiguousarray(np.asarray(inputs["x"], np.float32))
    B, C, T, F = x.shape
    assert (B, C, F) == (4, 2, 1025), (B, C, F)
    N_CORES = 8
    TS = T // N_CORES

    A, const = _fold_matrix(
        inputs["pre_w"], inputs["pre_b"], inputs["post_w"], inputs["post_b"],
        inputs["idx"], inputs["melw"], inputs["mask"], inputs["ola_window"],
    )
    A = A.astype(np.float32)
    wins, offs, TW, touches, first_j, last_j, colors = _plan(A)

    # packed fp8 band tensor, scaled by 2^6
    ab = np.zeros((_P, TW), ml_dtypes.float8_e3m4)
    for j in range(_NJ):
        lo, hi = wins[j]
        f0, f1 = j * _R, min((j + 1) * _R, _F)
        blk = A[:, f0:f1, :, lo:hi]                   # (ci, fl, co, w)
        q = np.clip(blk * np.float32(2.0 ** _SCALE_BITS), -15.5, 15.5)
        # [p = 2*fl + ci, 2*(fo-lo) + co]
        t = q.transpose(1, 0, 3, 2).reshape(2 * (f1 - f0), 2 * (hi - lo))
        ab[0:2 * (f1 - f0), offs[j]:offs[j] + 2 * (hi - lo)] = \
            t.astype(ml_dtypes.float8_e3m4)

    key = (TW, tuple(wins), N_CORES)
    if key not in _PROGRAM_CACHE:
        _PROGRAM_CACHE[key] = _build_program(
            wins, offs, TW, touches, first_j, last_j, colors, N_CORES)
    nc = _PROGRAM_CACHE[key]

    # host-side cast: bf16(x * 2^-6); the 2^6 lives in ab
    xq = (x * np.float32(2.0 ** -_SCALE_BITS)).astype(ml_dtypes.bfloat16)

    in_maps = []
    for m in range(N_CORES):
        sl = xq[:, :, m * TS:(m + 1) * TS, :1024]     # (tch, ci, t, 1024)
        sl = sl.reshape(_TCH, _C, TS, 16, _R)         # (tch, ci, t, j, fl)
        xs_m = np.ascontiguousarray(
            sl.transpose(4, 1, 3, 0, 2)               # (fl, ci, j, tch, t)
        ).reshape(_P, 16 * _TCORE)
        x1_m = np.ascontiguousarray(
            xq[:, :, m * TS:(m + 1) * TS, 1024].transpose(1, 0, 2)
        ).reshape(_C, _TCORE)
        in_maps.append({"xs": xs_m, "x1": x1_m, "ab": ab})

    try:
        import antenv.axon_hooks  # noqa: F401
    except ImportError:
        import sys
        import types
        import antenv
        stub = types.ModuleType("antenv.axon_hooks")
        stub.get_axon_ntff_profile_hook = lambda: None
        stub.set_axon_ntff_profile_hook = lambda h: None
        sys.modules["antenv.axon_hooks"] = stub
        antenv.axon_hooks = stub

    from concourse.bass_utils import run_bass_kernel_spmd
    res = run_bass_kernel_spmd(nc, in_maps, core_ids=list(range(N_CORES)))
    globals()["_LAST_RESULT"] = res

    out = np.empty((B, C, T, F), np.float32)
    for m in range(N_CORES):
        ym = res.results[m]["y"].astype(np.float32).reshape(_P, _TCH, F, C)
        ym = ym.transpose(1, 3, 0, 2)                 # (b, c, t, f)
        out[:, :, m * TS:(m + 1) * TS, :] = ym
    if np.any(const):
        out += const.astype(np.float32)[None, :, None, :]
    return out


# revision 4
# speedup vs baseline: 1.3803x; 1.3803x over previous
"""BandSplit (gather -> per-band MLP -> scatter-add OLA -> /ola) on 8 TRN2 cores.

Strategy
--------
The whole reference computation is linear in x: fold everything into one
block-banded matrix A of shape (C*F, C*F) so that per (b, t) token
out = A^T vec(x) + const (const == 0 here; added on host regardless).
Data-parallel over the 4096 (b, t) tokens across 8 cores, 512 tokens each,
no cross-core communication.

Key structure (v3):
 - Contraction chunks are 64 f-rows x 2 channels interleaved on the 128
   partitions (p = 2*fl + ci).  Halving the row span shrinks each chunk's
   output window (band support), cutting total matmul columns from ~27.8k
   (128-row, per-channel chunks) to ~19.1k cycles, and both input channels
   ride one matmul.
 - The packed band matrix `ab` ships as fp8 e3m4 scaled by 2^6 with the
   2^-6 folded into the host-side bf16 cast of x, so PSUM holds true-scale
   outputs and drains are plain f32->f16 copies.  Mixed bf16(x) x fp8(A)
   matmuls run at the full 1 col/cycle rate.
 - Every dma_start blocks its issuing sequencer for ~600 ns (HWDGE
   descriptor generation for 128 partition lines), so DMAs are few and
   fat: ab in 2 slabs + xs in 4 chunk-pairs per ring, y stored bank-major
   so each PSUM bank ships as one (or two) contiguous stores.  Loads split
   across both HWDGE rings (ACT and SP) which issue in parallel.
 - A short junk-matmul burst bridges PE-boot -> first-data; the HAM
   clock-gate latches 2.4 GHz only after ~6 us of *continuous* PE
   activity, so the stream is ordered exactly in DMA arrival order
   (j-major) to stay gapless.
 - PSUM bank lifetimes are 2-colored per token chunk (8 banks total);
   each bank drains (DVE/ACT alternating) right after its last chunk and
   stores immediately.
"""

import numpy as np

_P = 128
_C = 2
_F = 1025
_R = 64                     # f-rows per contraction chunk (x2 ci = 128)
_NJ = 17                    # chunks: j<16 cover f in [64j,64j+64); j=16 -> f=1024
_TCH = 4                    # token chunks (of 128) per core
_TCORE = _TCH * _P          # 512 tokens per core
_PS_W = _C * _F             # 2050 output columns (col = 2*fo + co)
# PSUM banks: 512-col pieces; the last 2 cols ride a separate 2-col bank but
# share bank 3's slot in the output layout (width 514).
_BANKS = [(0, 512), (512, 1024), (1024, 1536), (1536, 2048), (2048, 2050)]
_LAYW = [512, 512, 512, 514]          # output-layout widths (bank4 merged into 3)
_LAYB = [0, 2048, 4096, 6144]         # block base: b*4*width
_YW = 6144 + 4 * 514                  # 8200
_SCALE_BITS = 6             # ab * 2^6 in fp8; x * 2^-6 in bf16


def _fold_matrix(pre_w, pre_b, post_w, post_b, idx, melw, mask, ola_window):
    """Fold the full reference computation into (A, const).

    A: (C, F, C, F) with out[co, fo] = sum_{ci, fi} x[ci, fi] * A[ci, fi, co, fo]
    const: (C, F) additive constant from the biases.
    """
    K, W = idx.shape
    C = _C
    F = ola_window.shape[0]

    pre_w = np.asarray(pre_w, np.float64)
    post_w = np.asarray(post_w, np.float64)
    pre_b = np.asarray(pre_b, np.float64)
    post_b = np.asarray(post_b, np.float64)
    wts = (np.asarray(melw, np.float64) * np.asarray(mask, np.float64))
    msk = np.asarray(mask, np.float64)
    idx = np.asarray(idx)

    M = np.einsum('kio,koj->kij', pre_w, post_w).reshape(K, W, C, W, C)
    vals = M * wts[:, :, None, None, None] * msk[:, None, None, :, None]

    fin = idx[:, :, None, None, None].astype(np.int64)
    fout = idx[:, None, None, :, None].astype(np.int64)
    cin = np.arange(C)[None, None, :, None, None]
    cout = np.arange(C)[None, None, None, None, :]
    flat = ((cin * F + fin) * C + cout) * F + fout
    A = np.bincount(
        np.broadcast_to(flat, vals.shape).ravel(), weights=vals.ravel(),
        minlength=C * F * C * F,
    ).reshape(C, F, C, F)
    A /= ola_window[None, None, None, :]

    bv = (np.einsum('ko,koj->kj', pre_b, post_w) + post_b).reshape(K, W, C)
    bv = bv * msk[:, :, None]
    cflat = (np.arange(C)[None, None, :] * F + idx[:, :, None]).astype(np.int64)
    const = np.bincount(
        np.broadcast_to(cflat, bv.shape).ravel(), weights=bv.ravel(),
        minlength=C * F,
    ).reshape(C, F)
    const /= ola_window[None, :]
    return A, const


def _plan(A):
    """Windows, packed offsets, bank touch lists and PSUM slot colors."""
    wins = []                   # j -> (lo, hi) in fo units
    for j in range(_NJ):
        f0, f1 = j * _R, min((j + 1) * _R, _F)
        blk = A[:, f0:f1, :, :]
        cols = (blk != 0).any(axis=(0, 1, 2))
        nzc = np.nonzero(cols)[0]
        assert len(nzc) > 0
        wins.append((int(nzc[0]), int(nzc[-1]) + 1))
    covered = np.zeros(_F, bool)
    for lo, hi in wins:
        covered[lo:hi] = True
    assert covered.all(), "window coverage hole"

    offs = {}
    tw = 0
    for j in range(_NJ):
        offs[j] = tw
        tw += (2 * (wins[j][1] - wins[j][0]) + 15) // 16 * 16

    touches = {}                # b -> ordered [(j, s, e)] in column units
    for j in range(_NJ):
        lo2, hi2 = 2 * wins[j][0], 2 * wins[j][1]
        for b, (bs, be) in enumerate(_BANKS):
            s, e = max(lo2, bs), min(hi2, be)
            if s < e:
                touches.setdefault(b, []).append((j, s, e))
    first_j = {b: t[0][0] for b, t in touches.items()}
    last_j = {b: t[-1][0] for b, t in touches.items()}

    colors = {}
    for b in sorted(touches):
        used = {colors[o] for o in colors
                if not (last_j[o] < first_j[b] or last_j[b] < first_j[o])}
        free = [c for c in "AB" if c not in used]
        assert free, f"PSUM slot coloring needs >2 colors at bank {b}"
        colors[b] = free[0]
    return wins, offs, tw, touches, first_j, last_j, colors


def _olay(b, tch):
    """Output-layout (base, width) for PSUM bank b, token chunk tch."""
    lb = min(b, 3)
    base = _LAYB[lb] + tch * _LAYW[lb]
    if b == 4:
        base += 512
    return base


_PROGRAM_CACHE = {}


def _build_program(wins, offs, TW, touches, first_j, last_j, colors, n_cores):
    import concourse.tile as tile
    import concourse.mybir as mybir
    from concourse import bacc

    f32 = mybir.dt.float32
    bf16 = mybir.dt.bfloat16
    f16 = mybir.dt.float16
    f8e3 = mybir.dt.float8e3
    P = _P
    XCOLS = 16 * _TCORE          # 8192 cols: j*512 + tch*128 + tok  (j<16)
    W16 = 2 * (wins[16][1] - wins[16][0])
    AB_SPLIT = offs[8]           # ab slab split: j0-7 | j8-16

    nc = bacc.Bacc("TRN2", target_bir_lowering=False, debug=False,
                   num_devices=n_cores)
    xs = nc.dram_tensor("xs", [P, XCOLS], bf16, kind="ExternalInput")
    x1 = nc.dram_tensor("x1", [2, _TCORE], bf16, kind="ExternalInput")
    ab = nc.dram_tensor("ab", [P, TW], f8e3, kind="ExternalInput")
    # y: bank-major f16; block b at _LAYB[b], piece (b, tch) at _olay(b, tch)
    y = nc.dram_tensor("y", [P, _YW], f16, kind="ExternalOutput")

    with tile.TileContext(nc) as tc:
        with (
            tc.tile_pool(name="apool", bufs=1) as apool,
            tc.tile_pool(name="xpool", bufs=1) as xpool,
            tc.tile_pool(name="opool", bufs=1) as opool,
            tc.tile_pool(name="jpool", bufs=1) as jpool,
            tc.tile_pool(name="pspool", bufs=1, space="PSUM") as pspool,
        ):
            abig = apool.tile([P, TW], f8e3, name="abig")
            xbig = xpool.tile([P, XCOLS], bf16, name="xbig")
            t1 = jpool.tile([2, _TCORE], bf16, name="t1")
            junk = jpool.tile([P, P], bf16, name="junk")
            ot = opool.tile([P, _YW], f16, name="ot")

            nc.vector.memset(junk[:], 0.0)

            # Loads: both HWDGE rings issue in parallel (~600 ns per
            # dma_start on the issuing sequencer).  xs ships in 4-chunk
            # pairs [128, 1024] (2 KB lines); ab in two slabs.  Order on
            # each ring matches the stream's consumption order.
            def xs_pair(eng, p):
                eng.dma_start(xbig[:, p * 1024:(p + 1) * 1024],
                              xs[:, p * 1024:(p + 1) * 1024])

            nc.sync.dma_start(t1[:], x1[:])
            nc.scalar.dma_start(abig[:, :AB_SPLIT], ab[:, :AB_SPLIT])
            xs_pair(nc.sync, 0)          # j0, j1
            xs_pair(nc.scalar, 1)        # j2, j3
            xs_pair(nc.sync, 2)          # j4, j5
            nc.scalar.dma_start(abig[:, AB_SPLIT:], ab[:, AB_SPLIT:])
            xs_pair(nc.sync, 3)          # j6, j7
            xs_pair(nc.scalar, 4)        # j8, j9
            xs_pair(nc.sync, 5)          # j10, j11
            xs_pair(nc.scalar, 6)        # j12, j13
            xs_pair(nc.sync, 7)          # j14, j15

            # Warmup burst: bridge PE-boot -> first data, keep the HAM
            # continuous-activity window alive into the real stream.
            warm = pspool.tile([P, 512], f32, tag=f"{colors[0]}0", name="warm")
            for _ in range(20):
                nc.tensor.matmul(warm[:, :P], junk[:], junk[:],
                                 start=True, stop=True)

            cur = {}                    # (tch, b) -> PSUM tile
            drain_rr = [0]

            def drain(tch, b):
                base = _olay(b, tch)
                w = _BANKS[b][1] - _BANKS[b][0]
                t = cur.pop((tch, b))
                if drain_rr[0] % 2 == 0:
                    nc.vector.tensor_copy(ot[:, base:base + w], t[:])
                else:
                    nc.scalar.copy(ot[:, base:base + w], t[:])
                drain_rr[0] += 1

            def emit_mms(j, tch):
                lo2 = 2 * wins[j][0]
                if j < 16:
                    lhsT = xbig[:, j * _TCORE + tch * P:j * _TCORE + (tch + 1) * P]
                else:
                    lhsT = t1[:, tch * P:(tch + 1) * P]
                o = offs[j]
                for b, (bs, be) in enumerate(_BANKS):
                    s, e = max(lo2, bs), min(2 * wins[j][1], be)
                    if s >= e:
                        continue
                    if (tch, b) not in cur:
                        cur[(tch, b)] = pspool.tile(
                            [P, be - bs], f32, tag=f"{colors[b]}{tch}",
                            name=f"bk{b}_{tch}")
                    order = touches[b]
                    if j == 16:
                        at = abig[0:2, o + s - lo2:o + e - lo2]
                    else:
                        at = abig[:, o + s - lo2:o + e - lo2]
                    nc.tensor.matmul(
                        cur[(tch, b)][:, s - bs:e - bs],
                        lhsT, at,
                        start=(order[0][0] == j), stop=(order[-1][0] == j),
                    )

            # main stream: j-major matches arrival; banks finishing at j<15
            # drain all 4 tch then store as ONE contiguous bank-major DMA.
            for j in range(15):
                for tch in range(_TCH):
                    emit_mms(j, tch)
                for b in sorted(touches):
                    if last_j[b] == j:
                        for tch in range(_TCH):
                            drain(tch, b)
                        base = _LAYB[min(b, 3)]
                        w4 = 4 * _LAYW[min(b, 3)]
                        nc.sync.dma_start(y[:, base:base + w4],
                                          ot[:, base:base + w4])

            # tail: per token chunk, last two j's + drains; the merged
            # bank3+4 block stores in two halves (tch 0-1 on SP, 2-3 on ACT)
            tail_banks = sorted(b for b in touches if last_j[b] >= 15)
            base3 = _LAYB[3]
            for tch in range(_TCH):
                emit_mms(15, tch)
                emit_mms(16, tch)
                for b in tail_banks:
                    drain(tch, b)
                if tch == 1:
                    nc.sync.dma_start(y[:, base3:base3 + 1028],
                                      ot[:, base3:base3 + 1028])
                elif tch == 3:
                    nc.scalar.dma_start(y[:, base3 + 1028:base3 + 2056],
                                        ot[:, base3 + 1028:base3 + 2056])

    nc.compile()
    return nc


def kernel(**inputs):
    import ml_dtypes

    x = np.ascontiguousarray(np.asarray(inputs["x"], np.float32))
    B, C, T, F = x.shape
    assert (B, C, F) == (4, 2, 1025), (B, C, F)
    N_CORES = 8
    TS = T // N_CORES

    A, const = _fold_matrix(
        inputs["pre_w"], inputs["pre_b"], inputs["post_w"], inputs["post_b"],
        inputs["idx"], inputs["melw"], inputs["mask"], inputs["ola_window"],
    )
    A = A.astype(np.float32)
    wins, offs, TW, touches, first_j, last_j, colors = _plan(A)

    # packed fp8 band tensor, scaled by 2^6
    ab = np.zeros((_P, TW), ml_dtypes.float8_e3m4)
    for j in range(_NJ):
        lo, hi = wins[j]
        f0, f1 = j * _R, min((j + 1) * _R, _F)
        blk = A[:, f0:f1, :, lo:hi]                   # (ci, fl, co, w)
        q = np.clip(blk * np.float32(2.0 ** _SCALE_BITS), -15.5, 15.5)
        t = q.transpose(1, 0, 3, 2).reshape(2 * (f1 - f0), 2 * (hi - lo))
        ab[0:2 * (f1 - f0), offs[j]:offs[j] + 2 * (hi - lo)] = \
            t.astype(ml_dtypes.float8_e3m4)

    key = (TW, tuple(wins), N_CORES)
    if key not in _PROGRAM_CACHE:
        _PROGRAM_CACHE[key] = _build_program(
            wins, offs, TW, touches, first_j, last_j, colors, N_CORES)
    nc = _PROGRAM_CACHE[key]

    # host-side cast: bf16(x * 2^-6); the 2^6 lives in ab
    xq = (x * np.float32(2.0 ** -_SCALE_BITS)).astype(ml_dtypes.bfloat16)

    in_maps = []
    for m in range(N_CORES):
        sl = xq[:, :, m * TS:(m + 1) * TS, :1024]     # (tch, ci, t, 1024)
        sl = sl.reshape(_TCH, _C, TS, 16, _R)         # (tch, ci, t, j, fl)
        xs_m = np.ascontiguousarray(
            sl.transpose(4, 1, 3, 0, 2)               # (fl, ci, j, tch, t)
        ).reshape(_P, 16 * _TCORE)
        x1_m = np.ascontiguousarray(
            xq[:, :, m * TS:(m + 1) * TS, 1024].transpose(1, 0, 2)
        ).reshape(_C, _TCORE)
        in_maps.append({"xs": xs_m, "x1": x1_m, "ab": ab})

    try:
        import antenv.axon_hooks  # noqa: F401
    except ImportError:
        import sys
        import types
        import antenv
        stub = types.ModuleType("antenv.axon_hooks")
        stub.get_axon_ntff_profile_hook = lambda: None
        stub.set_axon_ntff_profile_hook = lambda h: None
        sys.modules["antenv.axon_hooks"] = stub
        antenv.axon_hooks = stub

    from concourse.bass_utils import run_bass_kernel_spmd
    res = run_bass_kernel_spmd(nc, in_maps, core_ids=list(range(N_CORES)))
    globals()["_LAST_RESULT"] = res

    out = np.empty((B, C, T, F), np.float32)
    for m in range(N_CORES):
        ym = res.results[m]["y"].astype(np.float32)   # (128, 8200) bank-major
        cols = np.empty((_P, _TCH, _PS_W), np.float32)
        for b in range(4):
            w = _LAYW[b]
            blk = ym[:, _LAYB[b]:_LAYB[b] + 4 * w].reshape(_P, _TCH, w)
            cols[:, :, 512 * b:512 * b + w] = blk
        ym4 = cols.reshape(_P, _TCH, F, C).transpose(1, 3, 0, 2)
        out[:, :, m * TS:(m + 1) * TS, :] = ym4
    if np.any(const):
        out += const.astype(np.float32)[None, :, None, :]
    return out
